# revision 25
# baseline (speedup 1.0000x reference)
"""Trainium2 Bass kernel for the segment-reduce cosine loss problem.

Reference computation (per sample b, S=32 labels):
  onehot[l,s] = (attributes[b,l] == s+1)
  seg_sum[s,:] = sum_l onehot[l,s] * text_feats[b,l,:]
  seg_mean     = seg_sum / count[s]
  cos[s] = <Vgs[b,s], seg_mean[s]> / max(|Vgs[b,s]| * |seg_mean[s]|, 1e-8)
  loss = mean_b (1 - mean_s cos[b,s]) = 1 - (sum_{b,s} cos) / (B*S)

Sharding: pure data parallel over batch. Each of the 8 cores processes 8
samples and outputs its [S, 8] cos matrix; the host sums them into the
scalar loss. Cosine similarity is invariant to positive scaling of
seg_mean, so the kernel works with seg_sum directly and never computes
the counts (the 1e-8 clamp is unreachable for this data distribution
either way: |V|*|seg_sum| is O(1e3)).

Per-core kernel (one NeuronCore, Tile framework on bacc):
  - attributes are cast to f32 and PE-transposed so each token position
    lands on a partition; all 8 onehot blocks [128, 32] for a sample are
    built in one DVE is_equal against an iota row (stride-0 broadcasts).
  - seg_sum runs on the PE in float32r (full-rate fp32 mode, tf32-like
    precision; the final scalar averages the noise away): lhsT = onehot
    chunk (stationary), rhs = text chunk [128, 512], accumulated over the
    8 L-chunks into PSUM [32, 512] x 2. Text streams in per-chunk 512 KB
    DMAs (24 tile buffers deep) and is the critical path: ~32 MB/core.
  - Vgs loads and |Vg|^2 norms (ACT Square with fused accum) are hoisted
    ahead of the text stream; per-sample epilogue computes <ss, Vg> (DVE
    mult from PSUM + reduce) and |ss|^2 (ACT Square + accum from PSUM).
  - cosine assembly (mult, sqrt, eps-clamp, reciprocal) is batched over
    all samples at [32, 8]; the Sqrt ACT table is pre-loaded at kernel
    start so the load is off the tail.
"""

import numpy as np

import concourse.mybir as mybir
import concourse.tile as tile
from concourse import bacc
from concourse.bass_utils import run_bass_kernel_spmd

B, L, D, S = 64, 1024, 1024, 32
N_CORES = 8
BPC = B // N_CORES        # samples per core
NCHUNK = L // 128         # L-chunks of 128 positions
EPS = 1e-8

F32 = mybir.dt.float32
F32R = mybir.dt.float32r
I32 = mybir.dt.int32
ALU = mybir.AluOpType
AXIS = mybir.AxisListType
ACTF = mybir.ActivationFunctionType


def build_bass():
    nc = bacc.Bacc(
        "TRN2", target_bir_lowering=False, debug=False, num_devices=N_CORES
    )
    attrs_d = nc.dram_tensor("attributes", [BPC, L], I32, kind="ExternalInput")
    text_d = nc.dram_tensor("text_feats", [BPC, L, D], F32R, kind="ExternalInput")
    vgs_d = nc.dram_tensor("Vgs", [BPC, S, D], F32, kind="ExternalInput")
    out_d = nc.dram_tensor("out", [S, BPC], F32, kind="ExternalOutput")

    with tile.TileContext(nc) as tc:
        with (
            tc.tile_pool(name="const", bufs=1) as const_pool,
            tc.tile_pool(name="text", bufs=24) as text_pool,
            tc.tile_pool(name="oh", bufs=4) as oh_pool,
            tc.tile_pool(name="work", bufs=2) as work_pool,
            tc.tile_pool(name="vgsp", bufs=BPC) as vgs_pool,
            tc.tile_pool(name="small", bufs=2) as small_pool,
            tc.tile_pool(name="psum", bufs=3, space="PSUM") as psum_pool,
            tc.tile_pool(name="psum1", bufs=1, space="PSUM") as psum1_pool,
        ):
            # ---- constants ----
            iota_s = const_pool.tile([128, S], F32, name="iota_s")
            nc.gpsimd.iota(
                iota_s[:], pattern=[[1, S]], base=1, channel_multiplier=0,
                allow_small_or_imprecise_dtypes=True,
            )
            warm = const_pool.tile([128, 1], F32, name="warm")
            nc.vector.memset(warm[:], 1.0)
            nc.scalar.sqrt(warm[:], warm[:])
            # 8x8 identity for the PE transpose of the attribute block
            idrow = const_pool.tile([BPC, BPC], F32, name="idrow")
            nc.gpsimd.iota(
                idrow[:], pattern=[[1, BPC]], base=0, channel_multiplier=0,
                allow_small_or_imprecise_dtypes=True,
            )
            idcol = const_pool.tile([BPC, 1], F32, name="idcol")
            nc.gpsimd.iota(
                idcol[:], pattern=[[0, 1]], base=0, channel_multiplier=1,
                allow_small_or_imprecise_dtypes=True,
            )
            ident = const_pool.tile([BPC, BPC], F32, name="ident")
            nc.vector.tensor_tensor(
                ident[:], idcol[:, 0:1].broadcast_to([BPC, BPC]), idrow[:],
                op=ALU.is_equal,
            )

            # ---- attribute prep: [BPC, L] i32 -> f32 -> transpose to [128, BPC*NCHUNK]
            attr_i = const_pool.tile([BPC, L], I32, name="attr_i")
            nc.scalar.dma_start(attr_i[:], attrs_d[:])
            attr_f = const_pool.tile([BPC, L], F32, name="attr_f")
            nc.vector.tensor_copy(attr_f[:], attr_i[:])
            psum_attr = psum1_pool.tile([128, NCHUNK * BPC], F32, tag="misc", name="psum_attr")
            for c in range(NCHUNK):
                # out[p, b] = attr_f[b, c*128 + p]
                nc.tensor.transpose(
                    psum_attr[:, c * BPC:(c + 1) * BPC],
                    attr_f[:, c * 128:(c + 1) * 128],
                    ident[:],
                )
            # permute (c, b) -> (b, c) while copying out of PSUM, so each
            # sample's NCHUNK attribute scalars are contiguous
            attr_sb = const_pool.tile([128, BPC * NCHUNK], F32, name="attr_sb")
            nc.vector.tensor_copy(
                attr_sb[:].rearrange("p (b c) -> p c b", c=NCHUNK),
                psum_attr[:].rearrange("p (c b) -> p c b", b=BPC),
            )

            # cos values per (attribute s = partition, sample b = column).
            # cos is scale-invariant in seg_mean, so seg_sum is used directly
            # and the 1/cnt normalization is skipped entirely.
            cos_all = const_pool.tile([32, BPC], F32, name="cos_all")
            num_all = const_pool.tile([S, BPC], F32, name="num_all")
            ns_parts = const_pool.tile([S, 2 * BPC], F32, name="ns_parts")

            # hoist all Vgs loads and |Vg|^2 norms ahead of the text stream
            nv_all = const_pool.tile([S, BPC], F32, name="nv_all")
            vgs_tiles = []
            for b in range(BPC):
                vg = vgs_pool.tile([S, D], F32, tag="vg", name=f"vg_{b}")
                nc.scalar.dma_start(vg[:], vgs_d[b])
                vgs_tiles.append(vg)
                sq3 = work_pool.tile([S, D], F32, tag="sq3", name=f"sq3_{b}")
                nc.scalar.activation(
                    sq3[:], vg[:], ACTF.Square, accum_out=nv_all[:, b:b + 1]
                )

            for b in range(BPC):
                # all NCHUNK onehot blocks for this sample in one DVE op:
                # oh_all[p, c, s] = (attr[b, c*128+p] == s+1)
                oh_all = oh_pool.tile([128, NCHUNK * S], F32R, tag="oh", name=f"oh_{b}")
                nc.vector.tensor_tensor(
                    oh_all[:].rearrange("p (c s) -> p c s", s=S),
                    attr_sb[:, b * NCHUNK:(b + 1) * NCHUNK]
                    .unsqueeze(2).broadcast_to([128, NCHUNK, S]),
                    iota_s[:].unsqueeze(1).broadcast_to([128, NCHUNK, S]),
                    op=ALU.is_equal,
                )
                psum_s0 = psum_pool.tile([32, 512], F32, tag="s0", name=f"ps0_{b}")
                psum_s1 = psum_pool.tile([32, 512], F32, tag="s1", name=f"ps1_{b}")
                for c in range(NCHUNK):
                    txc = text_pool.tile([128, D], F32R, tag="tx", name=f"tx_{b}_{c}")
                    rows = text_d[b, c * 128:(c + 1) * 128, :]
                    ohr = oh_all[:, c * S:(c + 1) * S]
                    st, sp = c == 0, c == NCHUNK - 1
                    if b == BPC - 1:
                        # last sample: split each chunk's DMA by D-half so the
                        # matmul on the first half hides its semaphore latency
                        # under the second half's transfer (shorter tail)
                        nc.sync.dma_start(txc[:, 0:512], rows[:, 0:512])
                        nc.sync.dma_start(txc[:, 512:D], rows[:, 512:D])
                    else:
                        nc.sync.dma_start(txc[:], rows)
                    nc.tensor.matmul(
                        psum_s0[:], ohr, txc[:, 0:512], start=st, stop=sp,
                    )
                    nc.tensor.matmul(
                        psum_s1[:], ohr, txc[:, 512:D], start=st, stop=sp,
                    )

                # ---- per-sample epilogue on partitions 0..31 ----
                vg = vgs_tiles[b]
                scr = work_pool.tile([S, D], F32, tag="scr", name=f"scr_{b}")
                sq2 = work_pool.tile([S, D], F32, tag="sq2", name=f"sq2_{b}")
                for h, ps in enumerate((psum_s0, psum_s1)):
                    # seg_sum * Vg (DVE) and seg_sum^2 with fused free-dim
                    # accumulation (ACT), both read straight out of PSUM
                    nc.vector.tensor_tensor(
                        scr[:, h * 512:(h + 1) * 512], ps[:],
                        vg[:, h * 512:(h + 1) * 512], op=ALU.mult,
                    )
                    nc.scalar.activation(
                        sq2[:, h * 512:(h + 1) * 512], ps[:], ACTF.Square,
                        accum_out=ns_parts[:, 2 * b + h:2 * b + h + 1],
                    )

                nc.vector.tensor_reduce(
                    num_all[:, b:b + 1], scr[:], axis=AXIS.X, op=ALU.add
                )

            # ---- batched cosine assembly over all samples [S, BPC] ----
            ns_all = small_pool.tile([S, BPC], F32, name="ns_all")
            nc.vector.tensor_reduce(
                ns_all[:], ns_parts[:].rearrange("s (b h) -> s b h", h=2),
                axis=AXIS.X, op=ALU.add,
            )
            prod = small_pool.tile([S, BPC], F32, name="prod")
            nc.vector.tensor_tensor(prod[:], ns_all[:], nv_all[:], op=ALU.mult)
            sq = small_pool.tile([S, BPC], F32, name="sq")
            nc.scalar.sqrt(sq[:], prod[:])
            den = small_pool.tile([S, BPC], F32, name="den")
            nc.vector.tensor_scalar(
                out=den[:], in0=sq[:], scalar1=float(EPS), scalar2=None,
                op0=ALU.max,
            )
            rec = small_pool.tile([S, BPC], F32, name="rec")
            nc.vector.reciprocal(rec[:], den[:])
            nc.vector.tensor_tensor(cos_all[:], num_all[:], rec[:], op=ALU.mult)

            nc.sync.dma_start(out_d[:], cos_all[:])

    nc.compile()
    return nc


_NC_CACHE = None


def _get_nc():
    global _NC_CACHE
    if _NC_CACHE is None:
        _NC_CACHE = build_bass()
    return _NC_CACHE


def kernel(attributes: np.ndarray, text_feats: np.ndarray, Vgs: np.ndarray) -> np.ndarray:
    assert attributes.shape == (B, L) and attributes.dtype == np.int32
    assert text_feats.shape == (B, L, D)
    assert Vgs.shape == (B, S, D)
    nc = _get_nc()
    in_maps = [
        {
            "attributes": np.ascontiguousarray(attributes[i * BPC:(i + 1) * BPC]),
            "text_feats": np.ascontiguousarray(text_feats[i * BPC:(i + 1) * BPC], dtype=np.float32),
            "Vgs": np.ascontiguousarray(Vgs[i * BPC:(i + 1) * BPC], dtype=np.float32),
        }
        for i in range(N_CORES)
    ]
    res = run_bass_kernel_spmd(nc, in_maps, core_ids=list(range(N_CORES)))
    total = sum(float(r["out"].sum()) for r in res.results)
    loss = 1.0 - total / (B * S)
    return np.asarray(loss, dtype=np.float32)
